# revision 6
# baseline (speedup 1.0000x reference)
"""Trainium2 Bass kernel for MinibatchDiscrimination — v2 (diag/query hybrid).

Reference op:
    h = (x @ w).reshape(B, U, O)                      # B=512, U=32, O=32
    D[i, o, j] = sum_u |h[i,u,o] - h[j,u,o]|          # pairwise L1 over units
    out[i, o]  = sum_j exp(-D[i,o,j])

Launch 1 (h = x @ w, uo-sharded, fp8): unchanged from the tuned baseline.

Launch 2 (pairwise): the elementwise max/abs stream is the bottleneck, and
DVE per-op cost is ~136ns fixed + 0.53ns/elem (TT, 2x mode) — so ops must be
WIDE. The pair set is partitioned two ways:

  * uo-chunks 0..5 (u 0..23) via the DIAGONAL form on DVE: for a distance
    offset delta in 1..32, ONE tensor_tensor max op [128, 6 chunks x 512 j]
    (FD=3072) computes max(h[:, a], h[:, a+d]) for all 512 column pairs at
    distance d = 32*core + delta. Per-core distance blocks come from the
    host-side rolls (hta rolled by 64c, htb by 96c), keeping the SPMD
    program uniform. 32 ops/core instead of 384 narrow ones.
  * uo-chunks 6..7 via the QUERY form on ACT (the only engine that can fold
    a per-partition scalar into an elementwise op): Abs(0.5*win - 0.5*h_i),
    128 ops of [128, 256], running fully concurrent with DVE.

  PE reduces both streams over u with a single shared stationary sel2
  (2.0 at p%32==o) in 4 column-quadrant tiles (concurrent matmuls), into
  full-bank [128, 512] diag pd tiles (4 distances x 32 o) and half-bank
  [128, 256] query pd tiles (4 queries x 32 o).

  ACT applies exp per pd bank (PSUM cannot be DMA'd directly, and an ACT
  copy costs the same as an exp); the bf16 exp tiles stream to HBM. The
  host applies the max-trick corrections (pd_diag = D_{0..5} + S6_a + S6_b
  -> multiply exp(S6_a)exp(S6_b); likewise S7 for the DVE-shuffled chunk-7
  queries) and multiplies the two partial exponentials per pair
  (exp(-(A+B)) = exp(-A) * exp(-B)), then folds each pair's term into both
  endpoint rows. Exps are deferred one group so a pd-gated exp never
  head-of-line blocks the ACT FIFO; query work is emitted before diag work
  per group so slow diag matmuls never block query matmuls in the PE FIFO.
"""

import os
import sys

import numpy as np

for _p in ("/opt/trn_rl_repo", "/root/.axon_site/_ro/trn_rl_repo"):
    if os.path.isdir(_p) and _p not in sys.path:
        sys.path.insert(0, _p)

import ml_dtypes  # noqa: E402

B = 512  # batch
D = 2048  # in features
U = 32  # units
O = 32  # units_out
UO = U * O  # 1024
NCORES = 8
BL = B // NCORES  # 64 own queries per core

KCH = D // 128  # 16 k-chunks
MCH = UO // 128  # 8 uo-chunks

# diag side (chunks 0..5 on DVE)
DCH = 6  # chunks reduced via the diagonal form
ND = 32  # distances per core
DG = ND // 4  # 8 groups of 4 distances (one PSUM bank each)
JD = B  # 512 j columns per diag op
JB = B + ND  # htb column count (needs j + delta reach)

# query side (chunks 6..7 on ACT)
QCH = (6, 7)
NSH = 24  # chunk-7 queries shuffled to DVE (max-form)
WQ = 256  # window width
WQW = BL + WQ  # 320: columns of chunks 6/7 actually read by the query side
NQ = 4  # queries per PSUM bank via PE column quadrants
NG = BL // NQ  # 16 query groups

_CACHE = {}
LAST_RESULTS = None  # BassKernelResults of the most recent run (for profiling)


def _build_h():
    """Launch-1 program: core c computes hT rows [128c, 128c+128) in bf16."""
    if "nc_h" in _CACHE:
        return _CACHE["nc_h"]

    from contextlib import ExitStack

    import concourse.mybir as mybir
    import concourse.tile as tile
    from concourse import bacc

    fp8 = mybir.dt.float8e4
    bf16 = mybir.dt.bfloat16
    f32 = mybir.dt.float32

    nc = bacc.Bacc(
        "TRN2", target_bir_lowering=False, debug=False, enable_asserts=False
    )
    xt_d = nc.dram_tensor("xt", [128, KCH * B], fp8, kind="ExternalInput")
    ws_d = nc.dram_tensor("ws", [128, KCH * 128], fp8, kind="ExternalInput")
    hts_d = nc.dram_tensor("hts", [128, B], bf16, kind="ExternalOutput")

    with tile.TileContext(nc) as tc, ExitStack() as ctx:
        pool = ctx.enter_context(tc.tile_pool(name="p", bufs=1))
        psum = ctx.enter_context(tc.tile_pool(name="ps", bufs=1, space="PSUM"))
        xt_sb = pool.tile([128, KCH * B], fp8, tag="xt")
        ws_sb = pool.tile([128, KCH * 128], fp8, tag="ws")
        # xt is column-quarter-major: [128, (quarter, k, 128)] so each
        # quarter's k-chain can start while later quarters still stream in
        QSZ = KCH * 128
        nc.scalar.dma_start(ws_sb[:], ws_d[:])
        nc.sync.dma_start(xt_sb[:, 0:QSZ], xt_d[:, 0:QSZ])
        nc.gpsimd.dma_start(xt_sb[:, QSZ : 2 * QSZ], xt_d[:, QSZ : 2 * QSZ])
        nc.sync.dma_start(xt_sb[:, 2 * QSZ : 3 * QSZ], xt_d[:, 2 * QSZ : 3 * QSZ])
        nc.gpsimd.dma_start(xt_sb[:, 3 * QSZ :], xt_d[:, 3 * QSZ :])
        # PE warm-up on junk data during the input DMA window
        junk = pool.tile([128, 128], bf16, tag="junk")
        nc.vector.memset(junk[:], 0.0)
        ps_w = psum.tile([128, 128], f32, name="ps_w", tag="ps_w")
        for _ in range(40):
            nc.tensor.matmul(ps_w[:], junk[:], junk[:], start=True, stop=True)
        ph = psum.tile([128, B], f32)
        for qt in range(4):
            cols = slice(qt * 128, qt * 128 + 128)
            for k in range(KCH):
                nc.tensor.matmul(
                    ph[:, cols],
                    ws_sb[:, k * 128 : (k + 1) * 128],
                    xt_sb[:, qt * QSZ + k * 128 : qt * QSZ + k * 128 + 128],
                    start=(k == 0),
                    stop=(k == KCH - 1),
                )
            hts = pool.tile([128, 128], bf16, tag=f"hts{qt}", name=f"hts{qt}")
            nc.scalar.activation(
                hts[:], ph[:, cols], mybir.ActivationFunctionType.Copy, scale=0.0625
            )
            nc.sync.dma_start(hts_d[:, cols], hts[:])

    nc.compile()
    _CACHE["nc_h"] = nc
    return nc


def _build_main():
    """Launch-2 (pairwise) SPMD program: diag on DVE + query on ACT."""
    if "nc2" in _CACHE:
        return _CACHE["nc2"]

    from contextlib import ExitStack

    import concourse.mybir as mybir
    import concourse.tile as tile
    from concourse import bacc

    bf16 = mybir.dt.bfloat16
    f32 = mybir.dt.float32
    AF = mybir.ActivationFunctionType
    AO = mybir.AluOpType

    nc = bacc.Bacc(
        "TRN2", target_bir_lowering=False, debug=False, enable_asserts=False
    )

    hta_d = nc.dram_tensor("hta", [128, DCH * JD + 2 * WQW], bf16, kind="ExternalInput")
    htb_d = nc.dram_tensor("htb", [128, DCH * JB], bf16, kind="ExternalInput")
    sel_d = nc.dram_tensor("sel", [128, 64], bf16, kind="ExternalInput")
    edd_d = nc.dram_tensor("edd", [128, DG * JD], bf16, kind="ExternalOutput")
    edq_d = nc.dram_tensor("edq", [128, NG * WQ], bf16, kind="ExternalOutput")

    with tile.TileContext(nc) as tc, ExitStack() as ctx:
        persist = ctx.enter_context(tc.tile_pool(name="persist", bufs=1))
        ad_pool = ctx.enter_context(tc.tile_pool(name="ad", bufs=8))
        aq_pool = ctx.enter_context(tc.tile_pool(name="aq", bufs=32))
        psw_pool = ctx.enter_context(tc.tile_pool(name="psw", bufs=1, space="PSUM"))
        pdd_pool = ctx.enter_context(tc.tile_pool(name="pdd", bufs=4, space="PSUM"))
        pdq_pool = ctx.enter_context(tc.tile_pool(name="pdq", bufs=3, space="PSUM"))
        ed_pool = ctx.enter_context(tc.tile_pool(name="ed", bufs=3))
        eq_pool = ctx.enter_context(tc.tile_pool(name="eq", bufs=3))

        sel_sb = persist.tile([128, 64], bf16, tag="sel")
        nc.gpsimd.dma_start(sel_sb[:], sel_d[:])
        sel2 = sel_sb[:, 32:64]

        # input DMAs spread over four queues; diag chunks (0..5) first
        hta = persist.tile([128, DCH * JD + 2 * WQW], bf16, tag="hta")
        htb = persist.tile([128, DCH * JB], bf16, tag="htb")
        nc.sync.dma_start(hta[:, 0:JD], hta_d[:, 0:JD])
        nc.gpsimd.dma_start(htb[:, 0:JB], htb_d[:, 0:JB])
        nc.scalar.dma_start(hta[:, 6 * JD :], hta_d[:, 6 * JD :])
        nc.sync.dma_start(hta[:, JD : 3 * JD], hta_d[:, JD : 3 * JD])
        nc.gpsimd.dma_start(htb[:, JB : 4 * JB], htb_d[:, JB : 4 * JB])
        nc.scalar.dma_start(htb[:, 4 * JB : 6 * JB], htb_d[:, 4 * JB : 6 * JB])
        nc.sync.dma_start(hta[:, 3 * JD : 6 * JD], hta_d[:, 3 * JD : 6 * JD])

        # PE warm-up on junk during the DMA window (HAM clock gate)
        junk = persist.tile([128, 128], bf16, tag="junk")
        nc.vector.memset(junk[:], 0.0)
        ps_w = psw_pool.tile([128, 128], f32, name="ps_w", tag="ps_w")
        for _ in range(48):
            nc.tensor.matmul(ps_w[:], junk[:], junk[:], start=True, stop=True)

        # chunks 6/7 are stored trimmed to WQW columns right after chunk 5
        qbase = {6: DCH * JD, 7: DCH * JD + WQW}
        # ACT scalars: hbN[m][:, i] = -h_i/2 for the Abs form
        hbN = {}
        for m in QCH:
            hbN[m] = persist.tile([128, BL], f32, tag=f"hbN{m}", name=f"hbN{m}")
            nc.scalar.activation(
                hbN[m][:], hta[:, qbase[m] : qbase[m] + BL], AF.Copy, scale=-0.5
            )
        # f32 chunk-7 query values for the DVE max-form shuffle (queries < NSH)
        hbP7 = persist.tile([128, NSH], f32, tag="hbP7")
        nc.scalar.activation(hbP7[:], hta[:, qbase[7] : qbase[7] + NSH], AF.Copy)

        hta3 = hta[:, 0 : DCH * JD].rearrange("p (m j) -> p m j", m=DCH)
        htb3 = htb.rearrange("p (m j) -> p m j", m=DCH)

        pdd_tiles = {}
        pdq_tiles = {}

        def emit_diag_group(G):
            pdd = pdd_pool.tile([128, JD], f32, name=f"pdd{G}", tag="pdd")
            pdd_tiles[G] = pdd
            for q in range(NQ):
                delta = 4 * G + q + 1
                ad = ad_pool.tile([128, DCH * JD], bf16, tag="ad", name=f"ad{G}_{q}")
                # first group: 3-chunk halves so DVE starts on partial DMA
                if G == 0:
                    splits = tuple((m, m + 1) for m in range(DCH))
                elif G == 1:
                    splits = ((0, 3), (3, 6))
                else:
                    splits = ((0, DCH),)
                for m0, m1 in splits:
                    nc.vector.tensor_tensor(
                        ad.rearrange("p (m j) -> p m j", m=DCH)[:, m0:m1, :],
                        hta3[:, m0:m1, :],
                        htb3[:, m0:m1, delta : delta + JD],
                        AO.max,
                    )
                for m in range(DCH):
                    nc.tensor.matmul(
                        pdd[O * q : O * (q + 1), :],
                        sel2,
                        ad[:, m * JD : (m + 1) * JD],
                        start=(m == 0),
                        stop=(m == DCH - 1),
                        tile_position=(0, O * q),
                    )

        def emit_diag_exp(G):
            pdd = pdd_tiles.pop(G)
            ed = ed_pool.tile([128, JD], bf16, tag="ed", name=f"ed{G}")
            nc.scalar.activation(ed[:], pdd[:], AF.Exp, scale=-1.0)
            nc.sync.dma_start(edd_d[:, G * JD : (G + 1) * JD], ed[:])

        def emit_query_pair(gp):
            # two query groups share one full PSUM bank -> one wide exp
            pdq = pdq_pool.tile([128, 2 * WQ], f32, name=f"pdq{gp}", tag="pdq")
            for half in range(2):
                g = 2 * gp + half
                cols = slice(half * WQ, half * WQ + WQ)
                for q in range(NQ):
                    i = NQ * g + q
                    for m in QCH:
                        aq = aq_pool.tile(
                            [128, WQ], bf16, tag="aq", name=f"aq{g}_{q}_{m}"
                        )
                        if m == 7 and i < NSH:
                            # DVE max-form (host applies the S7 correction)
                            nc.vector.tensor_scalar(
                                aq[:],
                                hta[:, qbase[m] + i + 1 : qbase[m] + i + 1 + WQ],
                                hbP7[:, i : i + 1],
                                None,
                                AO.max,
                            )
                        else:
                            nc.scalar.activation(
                                aq[:],
                                hta[:, qbase[m] + i + 1 : qbase[m] + i + 1 + WQ],
                                AF.Abs,
                                bias=hbN[m][:, i : i + 1],
                                scale=0.5,
                            )
                        nc.tensor.matmul(
                            pdq[O * q : O * (q + 1), cols],
                            sel2,
                            aq[:],
                            start=(m == QCH[0]),
                            stop=(m == QCH[-1]),
                            tile_position=(0, O * q),
                        )
            pdq_tiles[gp] = pdq

        def emit_query_exp(gp):
            pdq = pdq_tiles.pop(gp)
            eq = eq_pool.tile([128, 2 * WQ], bf16, tag="eq", name=f"eq{gp}")
            nc.scalar.activation(eq[:], pdq[:], AF.Exp, scale=-1.0)
            nc.gpsimd.dma_start(edq_d[:, gp * 2 * WQ : (gp + 1) * 2 * WQ], eq[:])

        for G in range(DG):
            emit_query_pair(G)
            emit_diag_group(G)
            if G >= 1:
                emit_query_exp(G - 1)
                emit_diag_exp(G - 1)
        emit_query_exp(DG - 1)
        emit_diag_exp(DG - 1)

    nc.compile()
    _strip_redundant_ldweights(nc)
    _CACHE["nc2"] = nc
    return nc


def _strip_redundant_ldweights(nc):
    """Drop PE weight reloads whose weights AP matches the already-loaded one."""
    import concourse.mybir as mybir

    PE = mybir.EngineType.PE
    keep_state = {"InstMatmult", "InstDrain", "InstEventSemaphore", "InstNop"}
    removed = 0
    for blk in nc.m.functions[0].blocks:
        insts = blk.instructions
        out = []
        loaded = {}
        for inst in insts:
            nm = type(inst).__name__
            if nm == "InstLdweights":
                ap = inst.ins[0]
                pos = tuple(inst.tile_position or (0, 0))
                key = (
                    ap.memref,
                    ap.offset,
                    tuple(map(tuple, ap.ap)),
                    str(ap.dtype),
                    inst.is_transpose,
                    inst.perf_mode,
                    tuple(inst.tile_size or ()),
                )
                si = inst.sync_info
                has_sync = si is not None and (
                    list(si.on_wait or []) or list(si.on_update or [])
                )
                if not has_sync and loaded.get(pos) == key:
                    removed += 1
                    continue
                if pos == (0, 0) and (inst.tile_size is None):
                    loaded = {}
                loaded[pos] = key
            elif nm not in keep_state and getattr(inst, "engine", None) == PE:
                loaded = {}
            out.append(inst)
        if removed:
            blk.instructions = out
    return removed


def _make_inputs_h(x: np.ndarray, w: np.ndarray):
    fp8 = ml_dtypes.float8_e4m3
    xt = np.ascontiguousarray(x.T).astype(fp8)  # [D, B]
    # column-quarter-major: [128, (quarter 4, k 16, col 128)]
    xt_p = np.ascontiguousarray(
        xt.reshape(KCH, 128, 4, 128).transpose(1, 2, 0, 3).reshape(128, KCH * B)
    )
    wb = (16.0 * w).astype(fp8)  # [D, UO] scaled into fp8 normal range
    ins = []
    for c in range(NCORES):
        ws = wb[:, 128 * c : 128 * (c + 1)]
        ws_p = np.ascontiguousarray(
            ws.reshape(KCH, 128, 128).transpose(1, 0, 2).reshape(128, KCH * 128)
        )
        ins.append({"xt": xt_p, "ws": ws_p})
    return ins


def _make_sel():
    sel = np.zeros((128, 64), dtype=ml_dtypes.bfloat16)
    p = np.arange(128)
    sel[p, 32 + p % O] = 2.0
    return sel


def _make_inputs_main(ht_global: np.ndarray):
    """Per-core hta (rolled 64c) and htb (rolled 96c) chunk-major buffers."""
    sel = _make_sel()
    ht3 = np.asarray(ht_global).reshape(MCH, 128, B)  # [m, p, j]
    ins = []
    for c in range(NCORES):
        ia = (np.arange(JD) + BL * c) % B
        ib = (np.arange(JB) + BL * c + ND * c) % B
        hta05 = ht3[:DCH, :, ia].transpose(1, 0, 2).reshape(128, DCH * JD)
        hta67 = ht3[DCH:, :, ia[:WQW]].transpose(1, 0, 2).reshape(128, 2 * WQW)
        hta = np.ascontiguousarray(np.concatenate([hta05, hta67], axis=1))
        htb = np.ascontiguousarray(
            ht3[:DCH, :, ib].transpose(1, 0, 2).reshape(128, DCH * JB)
        )
        ins.append({"hta": hta, "htb": htb, "sel": sel})
    return ins


def _assemble(results, ht_global) -> np.ndarray:
    """Host fold: join diag/query exp factors per pair, apply corrections.

    pdd[q, o, G, j] = sum_{u<24} 2*max(h[a], h[b]) = D06 + S6_a + S6_b where
    a = (j + 64c) % B, b = (a + d) % B, d = 32c + 4G + q + 1.
    pdq[q, o, g, k] = sum_{u>=24} |h_i - h_j| exact, i = 64c+4g+q, j = i+1+k.
    """
    hb = np.asarray(ht_global).astype(np.float64)  # [UO, B] bf16 values
    S6 = hb[: DCH * 128].reshape(DCH * 4, O, B).sum(axis=0)  # [O, B]
    F1 = np.zeros((O, B, 256), dtype=np.float64)  # exp(-D_{chunks 0..5})
    F2 = np.zeros((O, B, 256), dtype=np.float64)  # exp(-D_{chunks 6,7})
    jj = np.arange(JD)
    S7 = hb[7 * 128 :].reshape(4, O, B).sum(axis=0)  # [O, B] chunk-7 h sums
    eS6 = np.exp(S6)
    eS7 = np.exp(S7)
    for c in range(NCORES):
        edd = np.asarray(results[c]["edd"]).astype(np.float64)
        edd4 = edd.reshape(NQ, O, DG, JD)  # [q, o, G, j]
        a_idx = (jj + BL * c) % B
        for G in range(DG):
            for q in range(NQ):
                d = 32 * c + 4 * G + q + 1
                b_idx = (a_idx + d) % B
                F1[:, a_idx, d - 1] = (
                    edd4[q, :, G, :] * eS6[:, a_idx] * eS6[:, b_idx]
                )
        edq = np.asarray(results[c]["edq"]).astype(np.float64)
        edq4 = edq.reshape(NQ, O, NG, WQ)  # [q, o, g, k]
        for g in range(NG):
            for q in range(NQ):
                iloc = NQ * g + q
                i = (BL * c + iloc) % B
                F2[:, i, :] = edq4[q, :, g, :]
                if iloc < NSH:
                    # chunk-7 ran max-form: edq = exp(-(D67 + S7_i + S7_j))
                    j_idx = (i + 1 + np.arange(WQ)) % B
                    F2[:, i, :] *= eS7[:, i : i + 1] * eS7[:, j_idx]
    T = F1 * F2  # [O, a, d-1] = exp(-D_total) for pair (a, a+d)
    out = np.ones((B, O), dtype=np.float64)
    for d in range(1, 256):
        Td = T[:, :, d - 1]  # [O, a]
        out += Td.T
        out += np.roll(Td, d, axis=1).T
    T256 = T[:, 0:256, 255]  # pair {a, a+256}, counted from a < 256 only
    out[0:256, :] += T256.T
    out[256:512, :] += T256.T
    return out.astype(np.float32)


def kernel(x: np.ndarray, w: np.ndarray) -> np.ndarray:
    global LAST_RESULTS
    from concourse.bass_utils import run_bass_kernel_spmd

    nc_h = _build_h()
    nc2 = _build_main()
    res_h = run_bass_kernel_spmd(
        nc_h, _make_inputs_h(np.asarray(x), np.asarray(w)), list(range(NCORES))
    )
    ht_global = np.concatenate(
        [np.asarray(res_h.results[c]["hts"]) for c in range(NCORES)], axis=0
    )
    res = run_bass_kernel_spmd(
        nc2, _make_inputs_main(ht_global), list(range(NCORES))
    )
    LAST_RESULTS = (res_h, res)
    return _assemble(res.results, ht_global)


def _np_reference(x, w):
    h = (x @ w).reshape(B, U, O)
    diffs = h[:, :, :, None] - np.transpose(h, (1, 2, 0))[None, :, :, :]
    return np.exp(-np.abs(diffs).sum(axis=1)).sum(axis=-1)  # [B, O]


def _sim_core(nc, in_map, outs):
    from concourse.bass_interp import CoreSim

    sim = CoreSim(nc, trace=False)
    for name, arr in in_map.items():
        sim.tensor(name)[:] = arr
    sim.simulate(check_with_hw=False)
    return {o: sim.tensor(o).copy() for o in outs}


if __name__ == "__main__":
    # CoreSim checks; SCALE shrinks h so pairwise terms are O(1).
    SCALE = float(os.environ.get("KSIM_SCALE", "50"))
    rng = np.random.default_rng(0)
    x = (rng.normal(size=(B, D)) / SCALE).astype(np.float32)
    w = rng.uniform(-0.05, 0.05, size=(D, UO)).astype(np.float32)

    nc_h = _build_h()
    nc2 = _build_main()

    hts = []
    for c, im in enumerate(_make_inputs_h(x, w)):
        hts.append(_sim_core(nc_h, im, ["hts"])["hts"])
    ht_global = np.concatenate(hts, axis=0)
    h_ref = (x @ w).reshape(B, UO).T
    h_err = np.abs(ht_global.astype(np.float32) - h_ref).max() / max(
        np.abs(h_ref).max(), 1e-9
    )
    print(f"launch-1 simulated; h rel err (fp8 path): {h_err:.4g}")

    results = []
    for c, im in enumerate(_make_inputs_main(ht_global)):
        results.append(_sim_core(nc2, im, ["edd", "edq"]))
        print(f"core {c} simulated")
    got = _assemble(results, ht_global)

    h_sim = ht_global.astype(np.float32).T.reshape(B, U, O)
    diffs = h_sim[:, :, :, None] - np.transpose(h_sim, (1, 2, 0))[None, :, :, :]
    exp_ph2 = np.exp(-np.abs(diffs).sum(axis=1)).sum(axis=-1)
    err2 = np.abs(got - exp_ph2).max() / np.abs(exp_ph2).max()
    print("phase-2 rel err vs numpy-on-simulated-h:", err2)

    expected = _np_reference(x, w)
    err = np.abs(got - expected).max() / np.abs(expected).max()
    print("full-chain rel err vs fp32 numpy reference:", err)
    print(got[:2, :4])
    print(expected[:2, :4])


# revision 7
# speedup vs baseline: 1.0021x; 1.0021x over previous
"""Trainium2 Bass kernel for MinibatchDiscrimination — v2 (diag/query hybrid).

Reference op:
    h = (x @ w).reshape(B, U, O)                      # B=512, U=32, O=32
    D[i, o, j] = sum_u |h[i,u,o] - h[j,u,o]|          # pairwise L1 over units
    out[i, o]  = sum_j exp(-D[i,o,j])

Launch 1 (h = x @ w, uo-sharded, fp8): unchanged from the tuned baseline.

Launch 2 (pairwise): the elementwise max/abs stream is the bottleneck, and
DVE per-op cost is ~136ns fixed + 0.53ns/elem (TT, 2x mode) — so ops must be
WIDE. The pair set is partitioned two ways:

  * uo-chunks 0..5 (u 0..23) via the DIAGONAL form on DVE: for a distance
    offset delta in 1..32, ONE tensor_tensor max op [128, 6 chunks x 512 j]
    (FD=3072) computes max(h[:, a], h[:, a+d]) for all 512 column pairs at
    distance d = 32*core + delta. Per-core distance blocks come from the
    host-side rolls (hta rolled by 64c, htb by 96c), keeping the SPMD
    program uniform. 32 ops/core instead of 384 narrow ones.
  * uo-chunks 6..7 via the QUERY form on ACT (the only engine that can fold
    a per-partition scalar into an elementwise op): Abs(0.5*win - 0.5*h_i),
    128 ops of [128, 256], running fully concurrent with DVE.

  PE reduces both streams over u with a single shared stationary sel2
  (2.0 at p%32==o) in 4 column-quadrant tiles (concurrent matmuls), into
  full-bank [128, 512] diag pd tiles (4 distances x 32 o) and half-bank
  [128, 256] query pd tiles (4 queries x 32 o).

  ACT applies exp per pd bank (PSUM cannot be DMA'd directly, and an ACT
  copy costs the same as an exp); the bf16 exp tiles stream to HBM. The
  host applies the max-trick corrections (pd_diag = D_{0..5} + S6_a + S6_b
  -> multiply exp(S6_a)exp(S6_b); likewise S7 for the DVE-shuffled chunk-7
  queries) and multiplies the two partial exponentials per pair
  (exp(-(A+B)) = exp(-A) * exp(-B)), then folds each pair's term into both
  endpoint rows. Exps are deferred one group so a pd-gated exp never
  head-of-line blocks the ACT FIFO; query work is emitted before diag work
  per group so slow diag matmuls never block query matmuls in the PE FIFO.
"""

import os
import sys

import numpy as np

for _p in ("/opt/trn_rl_repo", "/root/.axon_site/_ro/trn_rl_repo"):
    if os.path.isdir(_p) and _p not in sys.path:
        sys.path.insert(0, _p)

import ml_dtypes  # noqa: E402

B = 512  # batch
D = 2048  # in features
U = 32  # units
O = 32  # units_out
UO = U * O  # 1024
NCORES = 8
BL = B // NCORES  # 64 own queries per core

KCH = D // 128  # 16 k-chunks
MCH = UO // 128  # 8 uo-chunks

# diag side (chunks 0..5 on DVE)
DCH = 6  # chunks reduced via the diagonal form
ND = 32  # distances per core
DG = ND // 4  # 8 groups of 4 distances (one PSUM bank each)
JD = B  # 512 j columns per diag op
JB = B + ND  # htb column count (needs j + delta reach)

# query side (chunks 6..7 on ACT)
QCH = (6, 7)
NSH = 24  # chunk-7 queries shuffled to DVE (max-form)
WQ = 256  # window width
WQW = BL + WQ  # 320: columns of chunks 6/7 actually read by the query side
NQ = 4  # queries per PSUM bank via PE column quadrants
NG = BL // NQ  # 16 query groups

_CACHE = {}
LAST_RESULTS = None  # BassKernelResults of the most recent run (for profiling)


def _build_h():
    """Launch-1 program: core c computes hT rows [128c, 128c+128) in bf16."""
    if "nc_h" in _CACHE:
        return _CACHE["nc_h"]

    from contextlib import ExitStack

    import concourse.mybir as mybir
    import concourse.tile as tile
    from concourse import bacc

    fp8 = mybir.dt.float8e4
    bf16 = mybir.dt.bfloat16
    f32 = mybir.dt.float32

    nc = bacc.Bacc(
        "TRN2", target_bir_lowering=False, debug=False, enable_asserts=False
    )
    xt_d = nc.dram_tensor("xt", [128, KCH * B], fp8, kind="ExternalInput")
    ws_d = nc.dram_tensor("ws", [128, KCH * 128], fp8, kind="ExternalInput")
    hts_d = nc.dram_tensor("hts", [128, B], bf16, kind="ExternalOutput")

    with tile.TileContext(nc) as tc, ExitStack() as ctx:
        pool = ctx.enter_context(tc.tile_pool(name="p", bufs=1))
        psum = ctx.enter_context(tc.tile_pool(name="ps", bufs=1, space="PSUM"))
        xt_sb = pool.tile([128, KCH * B], fp8, tag="xt")
        ws_sb = pool.tile([128, KCH * 128], fp8, tag="ws")
        # xt is column-quarter-major: [128, (quarter, k, 128)] so each
        # quarter's k-chain can start while later quarters still stream in
        QSZ = KCH * 128
        nc.scalar.dma_start(ws_sb[:], ws_d[:])
        nc.sync.dma_start(xt_sb[:, 0:QSZ], xt_d[:, 0:QSZ])
        nc.gpsimd.dma_start(xt_sb[:, QSZ : 2 * QSZ], xt_d[:, QSZ : 2 * QSZ])
        nc.sync.dma_start(xt_sb[:, 2 * QSZ : 3 * QSZ], xt_d[:, 2 * QSZ : 3 * QSZ])
        nc.gpsimd.dma_start(xt_sb[:, 3 * QSZ :], xt_d[:, 3 * QSZ :])
        # PE warm-up on junk data during the input DMA window
        junk = pool.tile([128, 128], bf16, tag="junk")
        nc.vector.memset(junk[:], 0.0)
        ps_w = psum.tile([128, 128], f32, name="ps_w", tag="ps_w")
        for _ in range(40):
            nc.tensor.matmul(ps_w[:], junk[:], junk[:], start=True, stop=True)
        ph = psum.tile([128, B], f32)
        for qt in range(4):
            cols = slice(qt * 128, qt * 128 + 128)
            for k in range(KCH):
                nc.tensor.matmul(
                    ph[:, cols],
                    ws_sb[:, k * 128 : (k + 1) * 128],
                    xt_sb[:, qt * QSZ + k * 128 : qt * QSZ + k * 128 + 128],
                    start=(k == 0),
                    stop=(k == KCH - 1),
                )
            hts = pool.tile([128, 128], bf16, tag=f"hts{qt}", name=f"hts{qt}")
            nc.scalar.activation(
                hts[:], ph[:, cols], mybir.ActivationFunctionType.Copy, scale=0.0625
            )
            nc.sync.dma_start(hts_d[:, cols], hts[:])

    nc.compile()
    _CACHE["nc_h"] = nc
    return nc


def _build_main():
    """Launch-2 (pairwise) SPMD program: diag on DVE + query on ACT."""
    if "nc2" in _CACHE:
        return _CACHE["nc2"]

    from contextlib import ExitStack

    import concourse.mybir as mybir
    import concourse.tile as tile
    from concourse import bacc

    bf16 = mybir.dt.bfloat16
    f32 = mybir.dt.float32
    AF = mybir.ActivationFunctionType
    AO = mybir.AluOpType

    nc = bacc.Bacc(
        "TRN2", target_bir_lowering=False, debug=False, enable_asserts=False
    )

    hta_d = nc.dram_tensor("hta", [128, DCH * JD + 2 * WQW], bf16, kind="ExternalInput")
    htb_d = nc.dram_tensor("htb", [128, DCH * JB], bf16, kind="ExternalInput")
    sel_d = nc.dram_tensor("sel", [128, 32], bf16, kind="ExternalInput")
    edd_d = nc.dram_tensor("edd", [128, DG * JD], bf16, kind="ExternalOutput")
    edq_d = nc.dram_tensor("edq", [128, NG * WQ], bf16, kind="ExternalOutput")

    with tile.TileContext(nc) as tc, ExitStack() as ctx:
        persist = ctx.enter_context(tc.tile_pool(name="persist", bufs=1))
        ad_pool = ctx.enter_context(tc.tile_pool(name="ad", bufs=8))
        aq_pool = ctx.enter_context(tc.tile_pool(name="aq", bufs=32))
        psw_pool = ctx.enter_context(tc.tile_pool(name="psw", bufs=1, space="PSUM"))
        pdd_pool = ctx.enter_context(tc.tile_pool(name="pdd", bufs=4, space="PSUM"))
        pdq_pool = ctx.enter_context(tc.tile_pool(name="pdq", bufs=3, space="PSUM"))
        ed_pool = ctx.enter_context(tc.tile_pool(name="ed", bufs=3))
        eq_pool = ctx.enter_context(tc.tile_pool(name="eq", bufs=3))

        sel_sb = persist.tile([128, 32], bf16, tag="sel")
        nc.gpsimd.dma_start(sel_sb[:], sel_d[:])
        sel2 = sel_sb[:]

        # input DMAs spread over four queues; diag chunks (0..5) first
        hta = persist.tile([128, DCH * JD + 2 * WQW], bf16, tag="hta")
        htb = persist.tile([128, DCH * JB], bf16, tag="htb")
        nc.sync.dma_start(hta[:, 0:JD], hta_d[:, 0:JD])
        nc.gpsimd.dma_start(htb[:, 0:JB], htb_d[:, 0:JB])
        nc.scalar.dma_start(hta[:, 6 * JD :], hta_d[:, 6 * JD :])
        nc.sync.dma_start(hta[:, JD : 3 * JD], hta_d[:, JD : 3 * JD])
        nc.gpsimd.dma_start(htb[:, JB : 4 * JB], htb_d[:, JB : 4 * JB])
        nc.scalar.dma_start(htb[:, 4 * JB : 6 * JB], htb_d[:, 4 * JB : 6 * JB])
        nc.sync.dma_start(hta[:, 3 * JD : 6 * JD], hta_d[:, 3 * JD : 6 * JD])

        # PE warm-up on junk during the DMA window (HAM clock gate)
        junk = persist.tile([128, 128], bf16, tag="junk")
        nc.vector.memset(junk[:], 0.0)
        ps_w = psw_pool.tile([128, 128], f32, name="ps_w", tag="ps_w")
        for _ in range(48):
            nc.tensor.matmul(ps_w[:], junk[:], junk[:], start=True, stop=True)

        # chunks 6/7 are stored trimmed to WQW columns right after chunk 5
        qbase = {6: DCH * JD, 7: DCH * JD + WQW}
        # ACT scalars: hbN[m][:, i] = -h_i/2 for the Abs form
        hbN = {}
        for m in QCH:
            hbN[m] = persist.tile([128, BL], f32, tag=f"hbN{m}", name=f"hbN{m}")
            nc.scalar.activation(
                hbN[m][:], hta[:, qbase[m] : qbase[m] + BL], AF.Copy, scale=-0.5
            )
        # f32 chunk-7 query values for the DVE max-form shuffle (queries < NSH)
        hbP7 = persist.tile([128, NSH], f32, tag="hbP7")
        nc.scalar.activation(hbP7[:], hta[:, qbase[7] : qbase[7] + NSH], AF.Copy)

        hta3 = hta[:, 0 : DCH * JD].rearrange("p (m j) -> p m j", m=DCH)
        htb3 = htb.rearrange("p (m j) -> p m j", m=DCH)

        pdd_tiles = {}
        pdq_tiles = {}

        def emit_diag_group(G):
            pdd = pdd_pool.tile([128, JD], f32, name=f"pdd{G}", tag="pdd")
            pdd_tiles[G] = pdd
            for q in range(NQ):
                delta = 4 * G + q + 1
                ad = ad_pool.tile([128, DCH * JD], bf16, tag="ad", name=f"ad{G}_{q}")
                # first group: 3-chunk halves so DVE starts on partial DMA
                if G == 0:
                    splits = tuple((m, m + 1) for m in range(DCH))
                elif G == 1:
                    splits = ((0, 3), (3, 6))
                else:
                    splits = ((0, DCH),)
                for m0, m1 in splits:
                    nc.vector.tensor_tensor(
                        ad.rearrange("p (m j) -> p m j", m=DCH)[:, m0:m1, :],
                        hta3[:, m0:m1, :],
                        htb3[:, m0:m1, delta : delta + JD],
                        AO.max,
                    )
                for m in range(DCH):
                    nc.tensor.matmul(
                        pdd[O * q : O * (q + 1), :],
                        sel2,
                        ad[:, m * JD : (m + 1) * JD],
                        start=(m == 0),
                        stop=(m == DCH - 1),
                        tile_position=(0, O * q),
                    )

        def emit_diag_exp(G):
            pdd = pdd_tiles.pop(G)
            ed = ed_pool.tile([128, JD], bf16, tag="ed", name=f"ed{G}")
            nc.scalar.activation(ed[:], pdd[:], AF.Exp, scale=-1.0)
            nc.sync.dma_start(edd_d[:, G * JD : (G + 1) * JD], ed[:])

        def emit_query_pair(gp):
            # two query groups share one full PSUM bank -> one wide exp
            pdq = pdq_pool.tile([128, 2 * WQ], f32, name=f"pdq{gp}", tag="pdq")
            for half in range(2):
                g = 2 * gp + half
                cols = slice(half * WQ, half * WQ + WQ)
                for q in range(NQ):
                    i = NQ * g + q
                    for m in QCH:
                        aq = aq_pool.tile(
                            [128, WQ], bf16, tag="aq", name=f"aq{g}_{q}_{m}"
                        )
                        if m == 7 and i < NSH:
                            # DVE max-form (host applies the S7 correction)
                            nc.vector.tensor_scalar(
                                aq[:],
                                hta[:, qbase[m] + i + 1 : qbase[m] + i + 1 + WQ],
                                hbP7[:, i : i + 1],
                                None,
                                AO.max,
                            )
                        else:
                            nc.scalar.activation(
                                aq[:],
                                hta[:, qbase[m] + i + 1 : qbase[m] + i + 1 + WQ],
                                AF.Abs,
                                bias=hbN[m][:, i : i + 1],
                                scale=0.5,
                            )
                        nc.tensor.matmul(
                            pdq[O * q : O * (q + 1), cols],
                            sel2,
                            aq[:],
                            start=(m == QCH[0]),
                            stop=(m == QCH[-1]),
                            tile_position=(0, O * q),
                        )
            pdq_tiles[gp] = pdq

        def emit_query_exp(gp):
            pdq = pdq_tiles.pop(gp)
            eq = eq_pool.tile([128, 2 * WQ], bf16, tag="eq", name=f"eq{gp}")
            nc.scalar.activation(eq[:], pdq[:], AF.Exp, scale=-1.0)
            nc.gpsimd.dma_start(edq_d[:, gp * 2 * WQ : (gp + 1) * 2 * WQ], eq[:])

        for G in range(DG):
            emit_query_pair(G)
            emit_diag_group(G)
            if G >= 1:
                emit_query_exp(G - 1)
                emit_diag_exp(G - 1)
        emit_query_exp(DG - 1)
        emit_diag_exp(DG - 1)

    nc.compile()
    _strip_redundant_ldweights(nc)
    _CACHE["nc2"] = nc
    return nc


def _strip_redundant_ldweights(nc):
    """Drop PE weight reloads whose weights AP matches the already-loaded one."""
    import concourse.mybir as mybir

    PE = mybir.EngineType.PE
    keep_state = {"InstMatmult", "InstDrain", "InstEventSemaphore", "InstNop"}
    removed = 0
    for blk in nc.m.functions[0].blocks:
        insts = blk.instructions
        out = []
        loaded = {}
        for inst in insts:
            nm = type(inst).__name__
            if nm == "InstLdweights":
                ap = inst.ins[0]
                pos = tuple(inst.tile_position or (0, 0))
                key = (
                    ap.memref,
                    ap.offset,
                    tuple(map(tuple, ap.ap)),
                    str(ap.dtype),
                    inst.is_transpose,
                    inst.perf_mode,
                    tuple(inst.tile_size or ()),
                )
                si = inst.sync_info
                has_sync = si is not None and (
                    list(si.on_wait or []) or list(si.on_update or [])
                )
                if not has_sync and loaded.get(pos) == key:
                    removed += 1
                    continue
                if pos == (0, 0) and (inst.tile_size is None):
                    loaded = {}
                loaded[pos] = key
            elif nm not in keep_state and getattr(inst, "engine", None) == PE:
                loaded = {}
            out.append(inst)
        if removed:
            blk.instructions = out
    return removed


def _make_inputs_h(x: np.ndarray, w: np.ndarray):
    fp8 = ml_dtypes.float8_e4m3
    xt = np.ascontiguousarray(x.T).astype(fp8)  # [D, B]
    # column-quarter-major: [128, (quarter 4, k 16, col 128)]
    xt_p = np.ascontiguousarray(
        xt.reshape(KCH, 128, 4, 128).transpose(1, 2, 0, 3).reshape(128, KCH * B)
    )
    wb = (16.0 * w).astype(fp8)  # [D, UO] scaled into fp8 normal range
    ins = []
    for c in range(NCORES):
        ws = wb[:, 128 * c : 128 * (c + 1)]
        ws_p = np.ascontiguousarray(
            ws.reshape(KCH, 128, 128).transpose(1, 0, 2).reshape(128, KCH * 128)
        )
        ins.append({"xt": xt_p, "ws": ws_p})
    return ins


def _make_sel():
    sel = np.zeros((128, 32), dtype=ml_dtypes.bfloat16)
    p = np.arange(128)
    sel[p, p % O] = 2.0
    return sel


def _make_inputs_main(ht_global: np.ndarray):
    """Per-core hta (rolled 64c) and htb (rolled 96c) chunk-major buffers."""
    sel = _make_sel()
    ht3 = np.asarray(ht_global).reshape(MCH, 128, B)  # [m, p, j]
    ins = []
    for c in range(NCORES):
        ia = (np.arange(JD) + BL * c) % B
        ib = (np.arange(JB) + BL * c + ND * c) % B
        hta05 = ht3[:DCH, :, ia].transpose(1, 0, 2).reshape(128, DCH * JD)
        hta67 = ht3[DCH:, :, ia[:WQW]].transpose(1, 0, 2).reshape(128, 2 * WQW)
        hta = np.ascontiguousarray(np.concatenate([hta05, hta67], axis=1))
        htb = np.ascontiguousarray(
            ht3[:DCH, :, ib].transpose(1, 0, 2).reshape(128, DCH * JB)
        )
        ins.append({"hta": hta, "htb": htb, "sel": sel})
    return ins


def _assemble(results, ht_global) -> np.ndarray:
    """Host fold: join diag/query exp factors per pair, apply corrections.

    pdd[q, o, G, j] = sum_{u<24} 2*max(h[a], h[b]) = D06 + S6_a + S6_b where
    a = (j + 64c) % B, b = (a + d) % B, d = 32c + 4G + q + 1.
    pdq[q, o, g, k] = sum_{u>=24} |h_i - h_j| exact, i = 64c+4g+q, j = i+1+k.
    """
    hb = np.asarray(ht_global).astype(np.float64)  # [UO, B] bf16 values
    S6 = hb[: DCH * 128].reshape(DCH * 4, O, B).sum(axis=0)  # [O, B]
    F1 = np.zeros((O, B, 256), dtype=np.float64)  # exp(-D_{chunks 0..5})
    F2 = np.zeros((O, B, 256), dtype=np.float64)  # exp(-D_{chunks 6,7})
    jj = np.arange(JD)
    S7 = hb[7 * 128 :].reshape(4, O, B).sum(axis=0)  # [O, B] chunk-7 h sums
    eS6 = np.exp(S6)
    eS7 = np.exp(S7)
    for c in range(NCORES):
        edd = np.asarray(results[c]["edd"]).astype(np.float64)
        edd4 = edd.reshape(NQ, O, DG, JD)  # [q, o, G, j]
        a_idx = (jj + BL * c) % B
        for G in range(DG):
            for q in range(NQ):
                d = 32 * c + 4 * G + q + 1
                b_idx = (a_idx + d) % B
                F1[:, a_idx, d - 1] = (
                    edd4[q, :, G, :] * eS6[:, a_idx] * eS6[:, b_idx]
                )
        edq = np.asarray(results[c]["edq"]).astype(np.float64)
        edq4 = edq.reshape(NQ, O, NG, WQ)  # [q, o, g, k]
        for g in range(NG):
            for q in range(NQ):
                iloc = NQ * g + q
                i = (BL * c + iloc) % B
                F2[:, i, :] = edq4[q, :, g, :]
                if iloc < NSH:
                    # chunk-7 ran max-form: edq = exp(-(D67 + S7_i + S7_j))
                    j_idx = (i + 1 + np.arange(WQ)) % B
                    F2[:, i, :] *= eS7[:, i : i + 1] * eS7[:, j_idx]
    T = F1 * F2  # [O, a, d-1] = exp(-D_total) for pair (a, a+d)
    out = np.ones((B, O), dtype=np.float64)
    for d in range(1, 256):
        Td = T[:, :, d - 1]  # [O, a]
        out += Td.T
        out += np.roll(Td, d, axis=1).T
    T256 = T[:, 0:256, 255]  # pair {a, a+256}, counted from a < 256 only
    out[0:256, :] += T256.T
    out[256:512, :] += T256.T
    return out.astype(np.float32)


def kernel(x: np.ndarray, w: np.ndarray) -> np.ndarray:
    global LAST_RESULTS
    from concourse.bass_utils import run_bass_kernel_spmd

    nc_h = _build_h()
    nc2 = _build_main()
    res_h = run_bass_kernel_spmd(
        nc_h, _make_inputs_h(np.asarray(x), np.asarray(w)), list(range(NCORES))
    )
    ht_global = np.concatenate(
        [np.asarray(res_h.results[c]["hts"]) for c in range(NCORES)], axis=0
    )
    res = run_bass_kernel_spmd(
        nc2, _make_inputs_main(ht_global), list(range(NCORES))
    )
    LAST_RESULTS = (res_h, res)
    return _assemble(res.results, ht_global)


def _np_reference(x, w):
    h = (x @ w).reshape(B, U, O)
    diffs = h[:, :, :, None] - np.transpose(h, (1, 2, 0))[None, :, :, :]
    return np.exp(-np.abs(diffs).sum(axis=1)).sum(axis=-1)  # [B, O]


def _sim_core(nc, in_map, outs):
    from concourse.bass_interp import CoreSim

    sim = CoreSim(nc, trace=False)
    for name, arr in in_map.items():
        sim.tensor(name)[:] = arr
    sim.simulate(check_with_hw=False)
    return {o: sim.tensor(o).copy() for o in outs}


if __name__ == "__main__":
    # CoreSim checks; SCALE shrinks h so pairwise terms are O(1).
    SCALE = float(os.environ.get("KSIM_SCALE", "50"))
    rng = np.random.default_rng(0)
    x = (rng.normal(size=(B, D)) / SCALE).astype(np.float32)
    w = rng.uniform(-0.05, 0.05, size=(D, UO)).astype(np.float32)

    nc_h = _build_h()
    nc2 = _build_main()

    hts = []
    for c, im in enumerate(_make_inputs_h(x, w)):
        hts.append(_sim_core(nc_h, im, ["hts"])["hts"])
    ht_global = np.concatenate(hts, axis=0)
    h_ref = (x @ w).reshape(B, UO).T
    h_err = np.abs(ht_global.astype(np.float32) - h_ref).max() / max(
        np.abs(h_ref).max(), 1e-9
    )
    print(f"launch-1 simulated; h rel err (fp8 path): {h_err:.4g}")

    results = []
    for c, im in enumerate(_make_inputs_main(ht_global)):
        results.append(_sim_core(nc2, im, ["edd", "edq"]))
        print(f"core {c} simulated")
    got = _assemble(results, ht_global)

    h_sim = ht_global.astype(np.float32).T.reshape(B, U, O)
    diffs = h_sim[:, :, :, None] - np.transpose(h_sim, (1, 2, 0))[None, :, :, :]
    exp_ph2 = np.exp(-np.abs(diffs).sum(axis=1)).sum(axis=-1)
    err2 = np.abs(got - exp_ph2).max() / np.abs(exp_ph2).max()
    print("phase-2 rel err vs numpy-on-simulated-h:", err2)

    expected = _np_reference(x, w)
    err = np.abs(got - expected).max() / np.abs(expected).max()
    print("full-chain rel err vs fp32 numpy reference:", err)
    print(got[:2, :4])
    print(expected[:2, :4])
